# revision 16
# baseline (speedup 1.0000x reference)
"""Distributed Trainium2 kernel for AdaptiveLinearWithChannel (MoE-routed
batched matmul):  out[t] = x[t] @ weight[indices[t]] + bias

Strategy (expert-parallel per the sharding hint): the tile dimension is
sharded 64-tiles-per-core across 8 NeuronCores; the indices gather is
resolved during host-side sharding (each core gets its 64 x-tiles plus
its 64 pre-gathered weight tiles), so routing is device-local and no
collectives are needed.

Quantized-precision pipeline (HBM traffic is the roofline — 8-bit
streams where the 2e-2 error gate allows):
  x   -> float8_e3m4 (x2 pre-scale; 4-mantissa fp8), 8 MiB/core
  w   -> bf16 pre-gathered,                          8 MiB/core
  out -> float8_e3m4 with a per-tile scale so[t] = 15.5/(7*sigma_est),
         dequantized on host (bias also added on host), 8 MiB/core
Total 24 MiB/core vs 40 MiB for the all-bf16 version. PE does
bf16(lhsT=w) x e3m4(rhs=xT) matmuls with exact f32 PSUM accumulation
(products have <=13-bit mantissas). l2 rel err vs the f32 reference:
1.889e-2 (gate 2e-2) — deterministic, validated bit-exact in numpy sim
against HW probes of the e3m4 decode and the engines' RNE f32->e3m4
conversion.

DMA-queue shape: the three DGE queues (x on Sync HWDGE, w on GpSimd
SWDGE, out on Scalar HWDGE) pace at ~186ns fixed cost per
partition-row packet, capping a queue at ~119 GB/s with 2 KiB rows.
SUP=4 tiles per super-tile makes every stream 4 KiB rows (~186 GB/s
cap) so the aggregate ~355 GB/s HBM limit is the only DMA constraint.
PSUM->SBUF copies are batched one [128,1024] op per tile (a PSUM bank
pair, j=0+j=1) with the per-tile dequant scale as a [128,1] scalar
operand: DVE takes even tiles, ACT odd tiles.  Semaphore count is kept
low (12) because every allocated semaphore adds ~0.1 us/engine to the
block-exit reset ladder.

Raw-bass pipeline (TileContext's multi-wait drain trips this walrus
build):
  SP     : HWDGE ring, x in-DMA per super-tile
  GpSimd : SWDGE ring, w in-DMA per super-tile + aux preamble
  PE     : 2x2 matmuls per tile into rotating PSUM bank pairs
  DVE    : PSUM -> SBUF e3m4 scale copy (even tiles)
  ACT    : same for odd tiles, plus HWDGE out-DMAs

A cheap full-coverage column-sum integrity check retries the rare
transient device corruption. Block exit skips gpsimd's SWDGE dge_drain
(its w-DMAs are all consumed by compute; ACT still drains the out-DMA
ring).
"""

import numpy as np
import ml_dtypes

import concourse.bass as bass
import concourse.mybir as mybir
from concourse.bass_utils import run_bass_kernel_spmd

BF16 = ml_dtypes.bfloat16
E3M4 = ml_dtypes.float8_e3m4

N_CORES = 8
NUM_TILES = 512
N_POINTS = 512          # free dim N of each matmul
D_IN = 256              # contraction, 2 chunks of 128
D_OUT = 256             # output partitions, 2 chunks of 128
CHANNELS = 1024
TPC = NUM_TILES // N_CORES   # 64 tiles per core
SUP = 4                      # tiles per super-tile (DMA batch; 4 KiB rows)
NSUP = TPC // SUP            # super-tiles per core
NBUF = 4                     # SBUF buffer sets (pipeline depth)
GROUPS_PER_SUP = SUP * 2     # psum groups per super-tile

E3M4_MAX = 15.5
OUT_MARGIN = 7.0             # out scale: so[t] = E3M4_MAX / (OUT_MARGIN*sigma)

_cache = {}


def _build_nc():
    bf = mybir.dt.bfloat16
    f32 = mybir.dt.float32
    e3 = mybir.dt.float8e3
    nc = bass.Bass()

    # x_dev[s, p, c, t2, f]    = x[4s+t2, f, 128c+p] * SX     (e3m4)
    # w_dev[s, p, c, t2, o]    = weight[idx[4s+t2], 128c+p, o] (bf16)
    # out_dev[s, po, t2, j, f] = out[4s+t2, f, 128j+po] * so[t]  (e3m4;
    #   t2-outermost so per-t2 out rows stay fully contiguous per partition)
    # aux_dev[p, t] = so[t]/SX  (psum multiplier)
    x_ext = nc.declare_dram_parameter("x", [NSUP, 128, 2, SUP, N_POINTS], e3, isOutput=False)
    w_ext = nc.declare_dram_parameter("w", [NSUP, 128, 2, SUP, D_OUT], bf, isOutput=False)
    aux_ext = nc.declare_dram_parameter("aux", [128, TPC], f32, isOutput=False)
    out_ext = nc.declare_dram_parameter("out", [NSUP, 128, SUP, 2, N_POINTS], e3, isOutput=True)

    import contextlib
    ctx = contextlib.ExitStack()
    x_sb = [ctx.enter_context(nc.sbuf_tensor(f"x_sb{i}", [128, 2, SUP, N_POINTS], e3)) for i in range(NBUF)]
    w_sb = [ctx.enter_context(nc.sbuf_tensor(f"w_sb{i}", [128, 2, SUP, D_OUT], bf)) for i in range(NBUF)]
    o_sb = [ctx.enter_context(nc.sbuf_tensor(f"o_sb{i}", [128, SUP, 2, N_POINTS], e3)) for i in range(NBUF)]
    aux_sb = ctx.enter_context(nc.sbuf_tensor("aux_sb", [128, TPC], f32))
    scr_sb = ctx.enter_context(nc.sbuf_tensor("scr_sb", [128, 1], f32))
    # 4 bank-pairs: tile t uses pair t%4, slice j*512..; one [128,1024]
    # copy per tile reads the whole pair
    psum = [ctx.enter_context(nc.psum_tensor(f"ps{i}", [128, 2 * N_POINTS], f32)) for i in range(4)]

    # DMA semaphores are per buffer-slot: a then_inc(sem, 16) lands as 16
    # independent +1s from the SDMA engines, so cumulative intermediate
    # thresholds on a sem with 2+ transfers in flight can fire on a mix of
    # them.  Per-slot sems are only ever waited at their full total (x+w
    # share one slot sem, waited at 32), and slot reuse quiesces the
    # previous use's incs first.  Compute sems (sem_pe/sem_dve/sem_acp:
    # single in-order producer) are safe cumulative.
    with ctx:
        with (
            contextlib.ExitStack() as sems,
            # skip GpSimd's expensive SWDGE dge_drain at block exit: the
            # w-DMAs it issued are consumed by compute long before the end,
            # and the ACT/SP engines still drain their own DGE queues (so
            # the final out-DMA completes before NEFF end)
            nc.Block(no_gpsimd_drain=True) as block,
        ):
            sem_xw = [sems.enter_context(nc.semaphore(f"sem_xw{b}")) for b in range(NBUF)]
            sem_o = [sems.enter_context(nc.semaphore(f"sem_o{b}")) for b in range(NBUF)]
            sem_aux = sems.enter_context(nc.semaphore("sem_aux"))
            sem_pe = sems.enter_context(nc.semaphore("sem_pe"))
            sem_dve = sems.enter_context(nc.semaphore("sem_dve"))
            sem_acp = sems.enter_context(nc.semaphore("sem_acp"))

            # per-tile copy completion: (sem, count) — DVE even, ACT odd
            def copy_done(t):
                return (sem_dve, t // 2 + 1) if t % 2 == 0 else (sem_acp, t // 2 + 1)

            @block.sync
            def _(sp):
                for s in range(NSUP):
                    if s >= NBUF:
                        # buffer s%NBUF free once PE finished super-tile s-NBUF
                        sp.wait_ge(sem_pe, GROUPS_PER_SUP * (s - NBUF + 1))
                    b = s % NBUF
                    sp.dma_start(x_sb[b][:], x_ext[s]).then_inc(sem_xw[b], 16)

            @block.gpsimd
            def _(gp):
                gp.dma_start(w_sb[0][:], w_ext[0]).then_inc(sem_xw[0], 16)
                gp.dma_start(aux_sb[:], aux_ext[:]).then_inc(sem_aux, 16)
                for s in range(1, NSUP):
                    if s >= NBUF:
                        gp.wait_ge(sem_pe, GROUPS_PER_SUP * (s - NBUF + 1))
                    b = s % NBUF
                    gp.dma_start(w_sb[b][:], w_ext[s]).then_inc(sem_xw[b], 16)

            @block.tensor
            def _(pe):
                for s in range(NSUP):
                    b, u = s % NBUF, s // NBUF
                    pe.wait_ge(sem_xw[b], 32 * (u + 1))
                    for t2 in range(SUP):
                        t = s * SUP + t2
                        if t >= 4:
                            # bank pair t%4 free once tile t-4's copy is done
                            csem, cnt = copy_done(t - 4)
                            pe.wait_ge(csem, cnt)
                        ps = psum[t % 4]
                        for j in range(2):
                            pe.matmul(ps[:, j * 512:(j + 1) * 512],
                                      w_sb[b][:, 0, t2, j * 128:(j + 1) * 128],
                                      x_sb[b][:, 0, t2, :], start=True, stop=False)
                            pe.matmul(ps[:, j * 512:(j + 1) * 512],
                                      w_sb[b][:, 1, t2, j * 128:(j + 1) * 128],
                                      x_sb[b][:, 1, t2, :], start=False, stop=True
                                      ).then_inc(sem_pe, 1)

            @block.vector
            def _(dve):
                dve.wait_ge(sem_aux, 16)
                for s in range(NSUP):
                    b, u = s % NBUF, s // NBUF
                    if s >= NBUF:
                        # o_sb buffer free once its previous out-DMA completed
                        dve.wait_ge(sem_o[b], 16 * u)
                    for t2 in (0, 2):
                        t = s * SUP + t2
                        dve.wait_ge(sem_pe, 2 * t + 2)
                        # out = psum * (so/SX)   -> e3m4, both j halves at once
                        dve.tensor_scalar_mul(o_sb[b][:, t2, :, :], psum[t % 4][:],
                                              aux_sb[:, t:t + 1]).then_inc(sem_dve, 1)

            @block.scalar
            def _(act):
                act.wait_ge(sem_aux, 16)
                # dummy activation: pulls the lazy 1.3us ACT_TABLE_LOAD into
                # the preamble instead of serializing it before the first
                # real PSUM copy
                act.activation(scr_sb[:], aux_sb[:, 0:1],
                               mybir.ActivationFunctionType.Copy)
                for s in range(NSUP):
                    b, u = s % NBUF, s // NBUF
                    if s >= NBUF:
                        act.wait_ge(sem_o[b], 16 * u)
                    for t2 in (1, 3):
                        t = s * SUP + t2
                        act.wait_ge(sem_pe, 2 * t + 2)
                        act.activation(o_sb[b][:, t2, :, :], psum[t % 4][:],
                                       mybir.ActivationFunctionType.Copy,
                                       scale=aux_sb[:, t:t + 1]).then_inc(sem_acp, 1)
                    # one out-DMA per super-tile (4 KiB rows).  Explicit sems
                    # even for ACT's own copies — dma_start only rings the
                    # DGE doorbell, its SBUF reads race the ACT pipeline
                    # otherwise.
                    act.wait_ge(sem_acp, 2 * (s + 1))
                    act.wait_ge(sem_dve, 2 * (s + 1))
                    act.dma_start(out_ext[s], o_sb[b][:]).then_inc(sem_o[b], 16)

    return nc


def _quant_x(x_f32):
    """Pick a pow2 pre-scale keeping |x*SX| comfortably under e3m4 max."""
    amax = float(np.abs(x_f32).max()) + 1e-30
    sx = 2.0 ** int(np.floor(np.log2(14.0 / amax)))
    return sx


def _pack_core(x_core_f32, w_gathered_bf16, so_core, sx):
    """Host-side repack of one core's shard into the device in_map."""
    x8 = (x_core_f32 * sx).astype(E3M4)                # [64, 512, 256]
    x_dev = np.ascontiguousarray(
        x8.reshape(NSUP, SUP, N_POINTS, 2, 128).transpose(0, 4, 3, 1, 2))
    w_dev = np.ascontiguousarray(
        w_gathered_bf16.reshape(NSUP, SUP, 2, 128, D_OUT).transpose(0, 3, 2, 1, 4))
    aux = np.ascontiguousarray(
        np.broadcast_to((so_core / sx)[None, :], (128, TPC))).astype(np.float32)
    return {"x": x_dev, "w": w_dev, "aux": aux}


def _unpack_core(out_dev, so_core):
    # [s, po, t2, j, f] -> [s, t2, f, j, po] -> [64, 512, 256], then dequant
    o = out_dev.transpose(0, 2, 4, 3, 1).reshape(TPC, N_POINTS, D_OUT).astype(np.float32)
    return o / so_core[:, None, None].astype(np.float32)


def _prepare(x, indices, weight, bias):
    """Shard + quantize all cores; returns (in_maps, so, colsum_ref, ref_norm)."""
    sx = _quant_x(x)
    weight_bf = weight.astype(BF16)
    wg_bf = weight_bf[indices]                          # [T, D_in, D_out]

    # per-tile out scale from input statistics (sigma_out ~ 16*sig_x*sig_w)
    sigx = np.sqrt((x.astype(np.float32) ** 2).mean(axis=(1, 2)))
    sigw = np.sqrt((wg_bf.astype(np.float32) ** 2).mean(axis=(1, 2)))
    sig_out = np.sqrt(D_IN) * sigx * sigw + 1e-30
    so = (E3M4_MAX / (OUT_MARGIN * sig_out)).astype(np.float32)   # [T]

    in_maps = []
    for k in range(N_CORES):
        sl = slice(k * TPC, (k + 1) * TPC)
        in_maps.append(_pack_core(x[sl], wg_bf[sl], so[sl], sx))

    # Integrity reference: column-sums are linear in the points axis, so
    # out_noBias[t].sum(axis=0) == (sum_p x_q[t]) @ w_bf[idx[t]] per tile
    # (up to e3m4 out-quant noise ~1.3e-2).  Full tile coverage at ~1% of
    # the compute — catches the rare transient device corruption.
    xq = (x * sx).astype(E3M4).astype(np.float32) / sx
    sxq = xq.sum(axis=1)                                          # [T, D_in]
    colsum_ref = np.einsum("ti,tio->to", sxq, wg_bf.astype(np.float32))
    ref_norm = np.linalg.norm(colsum_ref, axis=1) + 1e-6
    return in_maps, so, colsum_ref, ref_norm


def kernel(x, indices, weight, bias):
    x = np.asarray(x, dtype=np.float32)
    indices = np.asarray(indices).astype(np.int64)
    weight = np.asarray(weight, dtype=np.float32)
    bias = np.asarray(bias, dtype=np.float32)

    if "nc" not in _cache:
        _cache["nc"] = _build_nc()
    nc = _cache["nc"]

    in_maps, so, colsum_ref, ref_norm = _prepare(x, indices, weight, bias)

    # retry: the remote device occasionally hits a transient failure —
    # either an NRT error (exception) or, rarely, corrupted output blocks
    last_err = None
    out = None
    for attempt in range(4):
        try:
            res = run_bass_kernel_spmd(nc, in_maps, core_ids=list(range(N_CORES)))
        except Exception as e:  # noqa: BLE001
            last_err = e
            import time
            time.sleep(5.0 * (attempt + 1))
            continue
        cand = np.empty((NUM_TILES, N_POINTS, D_OUT), dtype=np.float32)
        for k in range(N_CORES):
            cand[k * TPC:(k + 1) * TPC] = _unpack_core(
                res.results[k]["out"], so[k * TPC:(k + 1) * TPC])
        per_tile_rel = np.linalg.norm(cand.sum(axis=1) - colsum_ref, axis=1) / ref_norm
        if per_tile_rel.max() < 5e-2:
            out = cand
            break
        last_err = RuntimeError(
            f"integrity check failed: max per-tile colsum rel err "
            f"{per_tile_rel.max():.3e} on tiles {np.where(per_tile_rel >= 5e-2)[0][:8]}")
    if out is None:
        raise last_err
    if np.any(bias):
        out += bias[0]
    return out


# revision 17
# speedup vs baseline: 1.0300x; 1.0300x over previous
"""Distributed Trainium2 kernel for AdaptiveLinearWithChannel (MoE-routed
batched matmul):  out[t] = x[t] @ weight[indices[t]] + bias

Strategy (expert-parallel per the sharding hint): the tile dimension is
sharded 64-tiles-per-core across 8 NeuronCores; the indices gather is
resolved during host-side sharding (each core gets its 64 x-tiles plus
its 64 pre-gathered weight tiles), so routing is device-local and no
collectives are needed.

Quantized-precision pipeline (HBM traffic is the roofline — 8-bit
streams where the 2e-2 error gate allows):
  x   -> float8_e3m4 (x2 pre-scale; 4-mantissa fp8), 8 MiB/core
  w   -> bf16 pre-gathered,                          8 MiB/core
  out -> float8_e3m4 with a per-tile scale so[t] = 15.5/(7*sigma_est),
         dequantized on host (bias also added on host), 8 MiB/core
Total 24 MiB/core vs 40 MiB for the all-bf16 version. PE does
bf16(lhsT=w) x e3m4(rhs=xT) matmuls with exact f32 PSUM accumulation
(products have <=13-bit mantissas). l2 rel err vs the f32 reference:
1.889e-2 (gate 2e-2) — deterministic, validated bit-exact in numpy sim
against HW probes of the e3m4 decode and the engines' RNE f32->e3m4
conversion.

DMA-queue shape: the three DGE queues (x on Sync HWDGE, w on GpSimd
SWDGE, out on Scalar HWDGE) pace at ~186ns fixed cost per
partition-row packet, capping a queue at ~119 GB/s with 2 KiB rows.
SUP=4 tiles per super-tile makes every stream 4 KiB rows (~186 GB/s
cap) so the aggregate ~355 GB/s HBM limit is the only DMA constraint.
PSUM->SBUF copies are batched one [128,1024] op per tile (a PSUM bank
pair, j=0+j=1) with the per-tile dequant scale as a [128,1] scalar
operand: DVE takes even tiles, ACT odd tiles.  Semaphore count is kept
low (12) because every allocated semaphore adds ~0.1 us/engine to the
block-exit reset ladder.

Raw-bass pipeline (TileContext's multi-wait drain trips this walrus
build):
  SP     : HWDGE ring, x in-DMA per super-tile
  GpSimd : SWDGE ring, w in-DMA per super-tile + aux preamble
  PE     : 2x2 matmuls per tile into rotating PSUM bank pairs
  DVE    : PSUM -> SBUF e3m4 scale copy (even tiles)
  ACT    : same for odd tiles, plus HWDGE out-DMAs

A cheap full-coverage column-sum integrity check retries the rare
transient device corruption. Block exit skips gpsimd's SWDGE dge_drain
(its w-DMAs are all consumed by compute; ACT still drains the out-DMA
ring).
"""

import numpy as np
import ml_dtypes

import concourse.bass as bass
import concourse.mybir as mybir
from concourse.bass_utils import run_bass_kernel_spmd

BF16 = ml_dtypes.bfloat16
E3M4 = ml_dtypes.float8_e3m4

N_CORES = 8
NUM_TILES = 512
N_POINTS = 512          # free dim N of each matmul
D_IN = 256              # contraction, 2 chunks of 128
D_OUT = 256             # output partitions, 2 chunks of 128
CHANNELS = 1024
TPC = NUM_TILES // N_CORES   # 64 tiles per core
SUP = 4                      # tiles per super-tile (DMA batch; 4 KiB rows)
NSUP = TPC // SUP            # super-tiles per core
NBUF = 4                     # SBUF buffer sets (pipeline depth)
GROUPS_PER_SUP = SUP * 2     # psum groups per super-tile

E3M4_MAX = 15.5
OUT_MARGIN = 7.0             # out scale: so[t] = E3M4_MAX / (OUT_MARGIN*sigma)

_cache = {}


def _build_nc(tail_split=True):
    bf = mybir.dt.bfloat16
    f32 = mybir.dt.float32
    e3 = mybir.dt.float8e3
    nc = bass.Bass()

    # x_dev[s, p, c, t2, f]    = x[4s+t2, f, 128c+p] * SX     (e3m4)
    # w_dev[s, p, c, t2, o]    = weight[idx[4s+t2], 128c+p, o] (bf16)
    # out_dev[s, po, t2, j, f] = out[4s+t2, f, 128j+po] * so[t]  (e3m4;
    #   t2-outermost so per-t2 out rows stay fully contiguous per partition)
    # aux_dev[p, t] = so[t]/SX  (psum multiplier)
    x_ext = nc.declare_dram_parameter("x", [NSUP, 128, 2, SUP, N_POINTS], e3, isOutput=False)
    w_ext = nc.declare_dram_parameter("w", [NSUP, 128, 2, SUP, D_OUT], bf, isOutput=False)
    aux_ext = nc.declare_dram_parameter("aux", [128, TPC], f32, isOutput=False)
    out_ext = nc.declare_dram_parameter("out", [NSUP, 128, SUP, 2, N_POINTS], e3, isOutput=True)

    import contextlib
    ctx = contextlib.ExitStack()
    x_sb = [ctx.enter_context(nc.sbuf_tensor(f"x_sb{i}", [128, 2, SUP, N_POINTS], e3)) for i in range(NBUF)]
    w_sb = [ctx.enter_context(nc.sbuf_tensor(f"w_sb{i}", [128, 2, SUP, D_OUT], bf)) for i in range(NBUF)]
    o_sb = [ctx.enter_context(nc.sbuf_tensor(f"o_sb{i}", [128, SUP, 2, N_POINTS], e3)) for i in range(NBUF)]
    aux_sb = ctx.enter_context(nc.sbuf_tensor("aux_sb", [128, TPC], f32))
    scr_sb = ctx.enter_context(nc.sbuf_tensor("scr_sb", [128, 1], f32))
    # 4 bank-pairs: tile t uses pair t%4, slice j*512..; one [128,1024]
    # copy per tile reads the whole pair
    psum = [ctx.enter_context(nc.psum_tensor(f"ps{i}", [128, 2 * N_POINTS], f32)) for i in range(4)]

    # DMA semaphores are per buffer-slot: a then_inc(sem, 16) lands as 16
    # independent +1s from the SDMA engines, so cumulative intermediate
    # thresholds on a sem with 2+ transfers in flight can fire on a mix of
    # them.  Per-slot sems are only ever waited at their full total (x+w
    # share one slot sem, waited at 32), and slot reuse quiesces the
    # previous use's incs first.  Compute sems (sem_pe/sem_dve/sem_acp:
    # single in-order producer) are safe cumulative.
    with ctx:
        with (
            contextlib.ExitStack() as sems,
            # skip GpSimd's expensive SWDGE dge_drain at block exit: the
            # w-DMAs it issued are consumed by compute long before the end,
            # and the ACT/SP engines still drain their own DGE queues (so
            # the final out-DMA completes before NEFF end)
            nc.Block(no_gpsimd_drain=True) as block,
        ):
            sem_xw = [sems.enter_context(nc.semaphore(f"sem_xw{b}")) for b in range(NBUF)]
            sem_o = [sems.enter_context(nc.semaphore(f"sem_o{b}")) for b in range(NBUF)]
            sem_aux = sems.enter_context(nc.semaphore("sem_aux"))
            sem_pe = sems.enter_context(nc.semaphore("sem_pe"))
            sem_dve = sems.enter_context(nc.semaphore("sem_dve"))
            sem_acp = sems.enter_context(nc.semaphore("sem_acp"))

            # per-tile copy completion: (sem, count) — DVE even, ACT odd
            def copy_done(t):
                return (sem_dve, t // 2 + 1) if t % 2 == 0 else (sem_acp, t // 2 + 1)

            @block.sync
            def _(sp):
                for s in range(NSUP):
                    if s >= NBUF:
                        # buffer s%NBUF free once PE finished super-tile s-NBUF
                        sp.wait_ge(sem_pe, GROUPS_PER_SUP * (s - NBUF + 1))
                    b = s % NBUF
                    sp.dma_start(x_sb[b][:], x_ext[s]).then_inc(sem_xw[b], 16)

            @block.gpsimd
            def _(gp):
                gp.dma_start(w_sb[0][:], w_ext[0]).then_inc(sem_xw[0], 16)
                gp.dma_start(aux_sb[:], aux_ext[:]).then_inc(sem_aux, 16)
                for s in range(1, NSUP):
                    if s >= NBUF:
                        gp.wait_ge(sem_pe, GROUPS_PER_SUP * (s - NBUF + 1))
                    b = s % NBUF
                    gp.dma_start(w_sb[b][:], w_ext[s]).then_inc(sem_xw[b], 16)

            @block.tensor
            def _(pe):
                for s in range(NSUP):
                    b, u = s % NBUF, s // NBUF
                    pe.wait_ge(sem_xw[b], 32 * (u + 1))
                    for t2 in range(SUP):
                        t = s * SUP + t2
                        if t >= 4:
                            # bank pair t%4 free once tile t-4's copy is done
                            csem, cnt = copy_done(t - 4)
                            pe.wait_ge(csem, cnt)
                        ps = psum[t % 4]
                        for j in range(2):
                            pe.matmul(ps[:, j * 512:(j + 1) * 512],
                                      w_sb[b][:, 0, t2, j * 128:(j + 1) * 128],
                                      x_sb[b][:, 0, t2, :], start=True, stop=False)
                            pe.matmul(ps[:, j * 512:(j + 1) * 512],
                                      w_sb[b][:, 1, t2, j * 128:(j + 1) * 128],
                                      x_sb[b][:, 1, t2, :], start=False, stop=True
                                      ).then_inc(sem_pe, 1)

            @block.vector
            def _(dve):
                dve.wait_ge(sem_aux, 16)
                for s in range(NSUP):
                    b, u = s % NBUF, s // NBUF
                    if s >= NBUF:
                        # o_sb buffer free once its previous out-DMA completed
                        dve.wait_ge(sem_o[b], 16 * u)
                    for t2 in (0, 2):
                        t = s * SUP + t2
                        dve.wait_ge(sem_pe, 2 * t + 2)
                        # out = psum * (so/SX)   -> e3m4, both j halves at once
                        dve.tensor_scalar_mul(o_sb[b][:, t2, :, :], psum[t % 4][:],
                                              aux_sb[:, t:t + 1]).then_inc(sem_dve, 1)

            @block.scalar
            def _(act):
                act.wait_ge(sem_aux, 16)
                # dummy activation: pulls the lazy 1.3us ACT_TABLE_LOAD into
                # the preamble instead of serializing it before the first
                # real PSUM copy
                act.activation(scr_sb[:], aux_sb[:, 0:1],
                               mybir.ActivationFunctionType.Copy)
                for s in range(NSUP):
                    b, u = s % NBUF, s // NBUF
                    if s >= NBUF:
                        act.wait_ge(sem_o[b], 16 * u)
                    for t2 in (1, 3):
                        t = s * SUP + t2
                        act.wait_ge(sem_pe, 2 * t + 2)
                        act.activation(o_sb[b][:, t2, :, :], psum[t % 4][:],
                                       mybir.ActivationFunctionType.Copy,
                                       scale=aux_sb[:, t:t + 1]).then_inc(sem_acp, 1)
                    # one out-DMA per super-tile (4 KiB rows).  Explicit sems
                    # even for ACT's own copies — dma_start only rings the
                    # DGE doorbell, its SBUF reads race the ACT pipeline
                    # otherwise.  The LAST super-tile goes out per-tile so
                    # the final drain chases each copy instead of waiting
                    # for all four.
                    if tail_split and s == NSUP - 1:
                        for t2 in range(SUP):
                            t = s * SUP + t2
                            csem, cnt = copy_done(t)
                            act.wait_ge(csem, cnt)
                            act.dma_start(out_ext[s][:, t2], o_sb[b][:, t2]
                                          ).then_inc(sem_o[b], 16)
                    else:
                        act.wait_ge(sem_acp, 2 * (s + 1))
                        act.wait_ge(sem_dve, 2 * (s + 1))
                        act.dma_start(out_ext[s], o_sb[b][:]).then_inc(sem_o[b], 16)

    return nc


def _quant_x(x_f32):
    """Pick a pow2 pre-scale keeping |x*SX| comfortably under e3m4 max."""
    amax = float(np.abs(x_f32).max()) + 1e-30
    sx = 2.0 ** int(np.floor(np.log2(14.0 / amax)))
    return sx


def _pack_core(x_core_f32, w_gathered_bf16, so_core, sx):
    """Host-side repack of one core's shard into the device in_map."""
    x8 = (x_core_f32 * sx).astype(E3M4)                # [64, 512, 256]
    x_dev = np.ascontiguousarray(
        x8.reshape(NSUP, SUP, N_POINTS, 2, 128).transpose(0, 4, 3, 1, 2))
    w_dev = np.ascontiguousarray(
        w_gathered_bf16.reshape(NSUP, SUP, 2, 128, D_OUT).transpose(0, 3, 2, 1, 4))
    aux = np.ascontiguousarray(
        np.broadcast_to((so_core / sx)[None, :], (128, TPC))).astype(np.float32)
    return {"x": x_dev, "w": w_dev, "aux": aux}


def _unpack_core(out_dev, so_core):
    # [s, po, t2, j, f] -> [s, t2, f, j, po] -> [64, 512, 256], then dequant
    o = out_dev.transpose(0, 2, 4, 3, 1).reshape(TPC, N_POINTS, D_OUT).astype(np.float32)
    return o / so_core[:, None, None].astype(np.float32)


def _prepare(x, indices, weight, bias):
    """Shard + quantize all cores; returns (in_maps, so, colsum_ref, ref_norm)."""
    sx = _quant_x(x)
    weight_bf = weight.astype(BF16)
    wg_bf = weight_bf[indices]                          # [T, D_in, D_out]

    # per-tile out scale from input statistics (sigma_out ~ 16*sig_x*sig_w)
    sigx = np.sqrt((x.astype(np.float32) ** 2).mean(axis=(1, 2)))
    sigw = np.sqrt((wg_bf.astype(np.float32) ** 2).mean(axis=(1, 2)))
    sig_out = np.sqrt(D_IN) * sigx * sigw + 1e-30
    so = (E3M4_MAX / (OUT_MARGIN * sig_out)).astype(np.float32)   # [T]

    in_maps = []
    for k in range(N_CORES):
        sl = slice(k * TPC, (k + 1) * TPC)
        in_maps.append(_pack_core(x[sl], wg_bf[sl], so[sl], sx))

    # Integrity reference: column-sums are linear in the points axis, so
    # out_noBias[t].sum(axis=0) == (sum_p x_q[t]) @ w_bf[idx[t]] per tile
    # (up to e3m4 out-quant noise ~1.3e-2).  Full tile coverage at ~1% of
    # the compute — catches the rare transient device corruption.
    xq = (x * sx).astype(E3M4).astype(np.float32) / sx
    sxq = xq.sum(axis=1)                                          # [T, D_in]
    colsum_ref = np.einsum("ti,tio->to", sxq, wg_bf.astype(np.float32))
    ref_norm = np.linalg.norm(colsum_ref, axis=1) + 1e-6
    return in_maps, so, colsum_ref, ref_norm


def kernel(x, indices, weight, bias):
    x = np.asarray(x, dtype=np.float32)
    indices = np.asarray(indices).astype(np.int64)
    weight = np.asarray(weight, dtype=np.float32)
    bias = np.asarray(bias, dtype=np.float32)

    if "nc" not in _cache:
        _cache["nc"] = _build_nc()
    nc = _cache["nc"]

    in_maps, so, colsum_ref, ref_norm = _prepare(x, indices, weight, bias)

    # retry: the remote device occasionally hits a transient failure —
    # either an NRT error (exception) or, rarely, corrupted output blocks
    last_err = None
    out = None
    for attempt in range(4):
        try:
            res = run_bass_kernel_spmd(nc, in_maps, core_ids=list(range(N_CORES)))
        except Exception as e:  # noqa: BLE001
            last_err = e
            import time
            time.sleep(5.0 * (attempt + 1))
            continue
        cand = np.empty((NUM_TILES, N_POINTS, D_OUT), dtype=np.float32)
        for k in range(N_CORES):
            cand[k * TPC:(k + 1) * TPC] = _unpack_core(
                res.results[k]["out"], so[k * TPC:(k + 1) * TPC])
        per_tile_rel = np.linalg.norm(cand.sum(axis=1) - colsum_ref, axis=1) / ref_norm
        if per_tile_rel.max() < 5e-2:
            out = cand
            break
        last_err = RuntimeError(
            f"integrity check failed: max per-tile colsum rel err "
            f"{per_tile_rel.max():.3e} on tiles {np.where(per_tile_rel >= 5e-2)[0][:8]}")
    if out is None:
        raise last_err
    if np.any(bias):
        out += bias[0]
    return out
